# revision 21
# baseline (speedup 1.0000x reference)
"""Bass/Tile TRN2 kernel for nn_LocalNodeAttentionHead.

Reference computation (per sample b):
    xi = x[:, :, t0]  (center frame)          (C, HW)
    xw = x reshaped                           (C, L)    L = T*H*W
    q  = Wq @ xi + bq                         (CI, HW)
    k  = Wk @ xw + bk                         (CI, L)
    v  = Wv @ xw + bv                         (L, CI)
    S  = q^T k  -> softmax over L             (HW, L)
    y  = softmax(S) @ v                       (CI, HW)
    out = Wo @ y + bo + xi                    (C, HW)

Algebraic restructure (host-folded weights; softmax is the only
nonlinearity, everything around it is linear):
    A  = Wq^T Wk   (C x C)     S = (A^T xi + r)^T xw,  r = Wk^T bq
    U  = Wo Wv     (C x C)     out = U (xw P^T) + (bo + Wo bv) + xi
This removes the k- and v-projections entirely (462M MACs each per
sample); bk drops by softmax shift invariance, bv/bo fold into the
residual carrier.

All device matmuls run in float16 (full PE rate, no free-dim
constraint); accumulation is fp32 in PSUM. The softmax 1/rowsum is
folded into the P^T transpose by multiplying against diag(rinv)
instead of the identity.

Distribution: pure data-parallel, 4 samples per core on 8 cores.
Scores run one sample ahead of transpose/z so the PE never waits on
the softmax chain. DMA triggers cost ~0.7us each on the queue engine
and per-core HBM bandwidth is ~300GB/s, so transfers are merged into
one trigger per tensor per sample, all on the otherwise-idle sync
queue, ordered so each sample's data lands just in time. Elementwise
work is balanced across scalar (exp, PSUM drains) and vector
(reduce/diag/residual/drains).
"""

import sys

sys.path.insert(0, "/opt/trn_rl_repo")

import numpy as np

import concourse.bass as bass
import concourse.tile as tile
from concourse import bacc, mybir

F32 = mybir.dt.float32
F16 = mybir.dt.float16
AF = mybir.ActivationFunctionType

B, C, T, H, W = 32, 512, 9, 14, 14
HWm = H * W  # 196
L = T * HWm  # 1764
CENT = (T // 2) * HWm  # 784, center-frame offset in L
NCORES = 8
BC = B // NCORES  # 4 samples per core

NCH = C // 128  # 4 chunks of the channel dim
LK = 441  # l-chunk for scores (4 chunks, <=512 moving free dim)
NLK = L // LK
LV = 126  # l-chunk for P^T / attention sum (14 chunks, <=128 partitions)
NLV = L // LV
MC = 98  # query-row chunk (2 chunks of HW=196)
NMC = HWm // MC
X0A = 980  # first x0 piece [0:980) covers the center frame and lc 0,1


def build_program():
    nc = bacc.Bacc("TRN2", target_bir_lowering=False, debug=False)

    # host-pre-tiled inputs; each load is a single fully-contiguous DMA.
    # Small constants are concatenated per-partition into single tensors so
    # one trigger covers each group:
    #   cst  = aT (2048) | rb-as-f16 (8) | ident (128) | xiq (3136)
    #   cst2 = xib (3136) | uT (2048)
    CST = 2048 + 8 + 128 + 3136
    CST2 = 3136 + 2048
    x = nc.dram_tensor("x", [BC, 128, NCH, L], F16, kind="ExternalInput").ap()
    xt = nc.dram_tensor("xt", [BC, LV, NLV, C], F16, kind="ExternalInput").ap()
    cst = nc.dram_tensor("cst", [128, CST], F16, kind="ExternalInput").ap()
    cst2 = nc.dram_tensor("cst2", [128, CST2], F16, kind="ExternalInput").ap()
    out = nc.dram_tensor("out", [BC, C, HWm], F16, kind="ExternalOutput").ap()

    with tile.TileContext(nc) as tc:
        with (
            tc.tile_pool(name="const", bufs=1) as const,
            tc.tile_pool(name="sb", bufs=1) as sb,
            tc.tile_pool(name="ps", bufs=8, space="PSUM") as ps,
        ):
            # ---- constants (q'-path first so the PE starts immediately) ----
            cst_sb = const.tile([128, CST], F16)
            nc.sync.dma_start(cst_sb[:], cst[:])
            aT_sb = cst_sb[:, 0:2048].rearrange("p (j c) -> p j c", j=NCH)
            rb_sb = cst_sb[:, 2048:2056].bitcast(F32)  # [128, NCH] f32
            id_sb = cst_sb[:, 2056:2184]
            xi_sb = cst_sb[:, 2184:CST].rearrange(
                "p (j s m) -> p j s m", j=NCH, s=BC
            )
            cst2_sb = const.tile([128, CST2], F16)
            xib_sb = cst2_sb[:, 0:3136].rearrange(
                "p (j s m) -> p j s m", j=NCH, s=BC
            )
            uT_sb = cst2_sb[:, 3136:CST2].rearrange("p (j c) -> p j c", j=NCH)
            q_sb = const.tile([128, NCH, BC * HWm], F16)

            def emit_qproj(pair):
                # q' for a sample pair, batched to N=392
                for ci in range(NCH):
                    qp = ps.tile([128, 2 * HWm], F32, tag="ps", name="qp")
                    for j in range(NCH):
                        nc.tensor.matmul(
                            qp[:],
                            aT_sb[:, j, ci * 128 : (ci + 1) * 128],
                            xi_sb[:, j, 2 * pair : 2 * pair + 2, :],
                            start=(j == 0),
                            stop=(j == NCH - 1),
                        )
                    nc.scalar.activation(
                        q_sb[
                            :, ci, 2 * pair * HWm : 2 * (pair + 1) * HWm
                        ],
                        qp[:],
                        AF.Identity,
                        bias=rb_sb[:, ci : ci + 1],
                    )

            # ---- per-sample pieces -----------------------------------------
            xw_t, xt_t, st_t, pt_t, rs_t, z_t = {}, {}, {}, {}, {}, {}

            def emit_load_x(s):
                xw = sb.tile([128, NCH, L], F16, tag="xw", bufs=2, name="xw")
                xw_t[s] = xw
                if s == 0:
                    # first piece covers the center frame (q') and lc 0,1
                    nc.sync.dma_start(xw[:, :, 0:X0A], x[s][:, :, 0:X0A])
                    nc.sync.dma_start(xw[:, :, X0A:L], x[s][:, :, X0A:L])
                else:
                    nc.sync.dma_start(xw[:], x[s])

            def emit_load_xt(s):
                xts = sb.tile([128, NLV, C], F16, tag="xt", bufs=2, name="xt")
                xt_t[s] = xts
                nc.sync.dma_start(xts[0:LV, :, :], xt[s])

            cmax_t = {}

            def emit_scores(s, midA=None, midB=None):
                # chunked=True (last sample): per-chunk PSUM reduces so the
                # row max is ready right after the last score chunk, keeping
                # the tail's softmax latency off the critical path
                chunked = s == BC - 1
                xw = xw_t[s]
                st = [
                    sb.tile([MC, L], F32, tag=f"s{mc}", bufs=2, name=f"s{mc}")
                    for mc in range(NMC)
                ]
                st_t[s] = st
                if chunked:
                    cmax_t[s] = [
                        sb.tile([MC, NLK], F32, tag=f"cm{mc}", bufs=1, name=f"cm{mc}")
                        for mc in range(NMC)
                    ]
                for lc in range(NLK):
                    if lc == 0 and midA is not None:
                        midA()
                    if lc == 1 and midB is not None:
                        midB()
                    for mc in range(NMC):
                        sp = ps.tile([MC, LK], F32, tag="ps", name="sp")
                        for cj in range(NCH):
                            nc.tensor.matmul(
                                sp[:],
                                q_sb[
                                    :,
                                    cj,
                                    s * HWm + mc * MC : s * HWm + (mc + 1) * MC,
                                ],
                                xw[:, cj, lc * LK : (lc + 1) * LK],
                                start=(cj == 0),
                                stop=(cj == NCH - 1),
                            )
                        if chunked:
                            nc.scalar.copy(
                                st[mc][:, lc * LK : (lc + 1) * LK], sp[:]
                            )
                            nc.vector.reduce_max(
                                cmax_t[s][mc][:, lc : lc + 1],
                                sp[:],
                                axis=mybir.AxisListType.X,
                            )
                        elif lc < NLK - 1:
                            nc.scalar.copy(
                                st[mc][:, lc * LK : (lc + 1) * LK], sp[:]
                            )
                        else:
                            nc.vector.tensor_copy(
                                st[mc][:, lc * LK : (lc + 1) * LK], sp[:]
                            )

            nm_t = {}

            def emit_negmax(s):
                # row max on vector (chunked variant reduces the tiny cmax)
                chunked = s == BC - 1
                nm = []
                for mc in range(NMC):
                    negmax = sb.tile([MC, 1], F32, tag="negmax", bufs=2, name="negmax")
                    nc.vector.reduce_max(
                        negmax[:],
                        cmax_t[s][mc][:] if chunked else st_t[s][mc][:],
                        axis=mybir.AxisListType.X,
                        negate=True,
                    )
                    nm.append(negmax)
                nm_t[s] = nm

            def emit_exp(s):
                # exp (unnormalized, max-shifted) -> f16 P with the row sum
                # accumulated
                pt, rs = [], []
                for mc in range(NMC):
                    p = sb.tile([MC, L], F16, tag=f"p{mc}", bufs=2, name=f"p{mc}")
                    rsum = sb.tile([MC, 1], F32, tag="rsum", bufs=2, name="rsum")
                    nc.scalar.activation(
                        p[:],
                        st_t[s][mc][:],
                        AF.Exp,
                        bias=nm_t[s][mc][:],
                        accum_out=rsum[:],
                    )
                    pt.append(p)
                    rs.append(rsum)
                pt_t[s] = pt
                rs_t[s] = rs

            def emit_tz(s):
                # P^T (normalized via diag(rinv)) and z = xw P^T, streamed
                # over l-chunk pairs; z accumulates C-major so no output
                # transpose is needed
                pt, xts = pt_t[s], xt_t[s]
                dg = []
                for mc in range(NMC):
                    rinv = sb.tile([MC, 1], F32, tag="rinv", bufs=2, name="rinv")
                    nc.vector.reciprocal(rinv[:], rs_t[s][mc][:])
                    d = sb.tile([MC, MC], F16, tag=f"dg{mc}", bufs=2, name=f"dg{mc}")
                    nc.vector.tensor_scalar_mul(d[:], id_sb[0:MC, 0:MC], rinv[:])
                    dg.append(d)
                z_ps = [
                    ps.tile([128, HWm], F32, tag="ps", name=f"z{cc}")
                    for cc in range(NCH)
                ]
                for lp in range(NLV // 2):
                    ptp = ps.tile([LV, 2 * HWm], F32, tag="ps", name="ptp")
                    for half in range(2):
                        lc = 2 * lp + half
                        for mc in range(NMC):
                            nc.tensor.matmul(
                                ptp[
                                    :,
                                    half * HWm + mc * MC : half * HWm + (mc + 1) * MC,
                                ],
                                pt[mc][:, lc * LV : (lc + 1) * LV],
                                dg[mc][:],
                                start=True,
                                stop=True,
                            )
                    ptsb = sb.tile(
                        [128, 2 * HWm], F16, tag="ptsb", bufs=3, name="ptsb"
                    )
                    if lp % 2 == 0:
                        nc.vector.tensor_copy(ptsb[0:LV, :], ptp[:])
                    else:
                        nc.scalar.copy(ptsb[0:LV, :], ptp[:])
                    for half in range(2):
                        lc = 2 * lp + half
                        for cc in range(NCH):
                            nc.tensor.matmul(
                                z_ps[cc][:],
                                xts[0:LV, lc, cc * 128 : (cc + 1) * 128],
                                ptsb[0:LV, half * HWm : (half + 1) * HWm],
                                start=(lc == 0),
                                stop=(lc == NLV - 1),
                            )
                # z -> sbuf f16 for the output projection (pair-batched)
                if s % 2 == 0:
                    z_t[s // 2] = sb.tile(
                        [128, NCH, 2 * HWm], F16, tag="z2", bufs=2, name="z2"
                    )
                z2 = z_t[s // 2]
                for cc in range(NCH):
                    nc.vector.tensor_copy(
                        z2[:, cc, (s % 2) * HWm : (s % 2 + 1) * HWm], z_ps[cc][:]
                    )

            def emit_out(pair):
                # output projection + residual (center frame + bo2, f16 const)
                z2 = z_t[pair]
                osb = sb.tile([128, NCH, 2 * HWm], F16, tag="osb", bufs=2, name="osb")
                for co in range(NCH):
                    op = ps.tile([128, 2 * HWm], F32, tag="ps", name="op")
                    for cj in range(NCH):
                        nc.tensor.matmul(
                            op[:],
                            uT_sb[:, cj, co * 128 : (co + 1) * 128],
                            z2[:, cj, :],
                            start=(cj == 0),
                            stop=(cj == NCH - 1),
                        )
                    nc.vector.tensor_add(
                        osb[:, co, :],
                        op[:],
                        xib_sb[:, co, 2 * pair : 2 * pair + 2, :],
                    )
                for ds in range(2):
                    nc.sync.dma_start(
                        out[2 * pair + ds].rearrange("(j p) m -> p j m", p=128),
                        osb[:, :, ds * HWm : (ds + 1) * HWm],
                    )

            # ---- schedule: scores one sample ahead of transpose/z; DMA
            # triggers ordered so each sample's data lands just in time -----
            emit_load_x(0)
            emit_load_x(1)
            emit_qproj(0)
            emit_qproj(1)
            emit_scores(0)
            emit_negmax(0)
            emit_exp(0)
            emit_load_xt(0)

            emit_scores(1)
            emit_load_x(2)
            emit_load_xt(1)
            emit_tz(0)

            nc.sync.dma_start(cst2_sb[:], cst2[:])
            emit_scores(
                2,
                midA=lambda: emit_negmax(1),
                midB=lambda: emit_exp(1),
            )
            emit_load_x(3)
            emit_load_xt(2)
            emit_tz(1)
            emit_out(0)

            emit_scores(
                3,
                midA=lambda: emit_negmax(2),
                midB=lambda: emit_exp(2),
            )
            emit_load_xt(3)
            emit_negmax(3)
            emit_exp(3)
            emit_tz(2)

            emit_tz(3)
            emit_out(1)

    nc.compile()
    return nc


_NC = None


def _get_program():
    global _NC
    if _NC is None:
        _NC = build_program()
    return _NC


def make_in_maps(inputs):
    x_window = np.ascontiguousarray(np.asarray(inputs["x_window"], dtype=np.float32))
    Wq = np.asarray(inputs["Wq"], dtype=np.float32)
    bq_ = np.asarray(inputs["bq"], dtype=np.float32)
    Wk = np.asarray(inputs["Wk"], dtype=np.float32)
    Wv = np.asarray(inputs["Wv"], dtype=np.float32)
    bv_ = np.asarray(inputs["bv"], dtype=np.float32)
    Wo = np.asarray(inputs["Wo"], dtype=np.float32)
    bo_ = np.asarray(inputs["bo"], dtype=np.float32)

    # host-folded weights (fp32, exact)
    A = Wq.T @ Wk  # (C, C): S = (A^T xi + r)^T xw
    r = Wk.T @ bq_
    U = Wo @ Wv  # (C, C): out = U (xw P^T) + bo2 + xi
    bo2 = bo_ + Wo @ bv_

    xw = x_window.reshape(B, C, L)
    # residual carrier: center frame + folded output bias
    xib_full = xw[:, :, CENT : CENT + HWm] + bo2[None, :, None]

    def tile_w(wt):  # (in, out) -> [128, NCH, out] partition-major
        return np.ascontiguousarray(
            wt.reshape(NCH, 128, -1).transpose(1, 0, 2).astype(np.float16)
        )

    aT_h = tile_w(A).reshape(128, -1)
    uT_h = tile_w(U.T).reshape(128, -1)
    rb_h = np.ascontiguousarray(
        r.reshape(NCH, 128).T.astype(np.float32)
    ).view(np.float16)  # [128, 8]
    id_h = np.eye(128, dtype=np.float16)
    in_maps = []
    for i in range(NCORES):
        xc = xw[i * BC : (i + 1) * BC]  # (BC, C, L)
        xiq_h = (
            xc[:, :, CENT : CENT + HWm]
            .reshape(BC, NCH, 128, HWm)
            .transpose(2, 1, 0, 3)
            .astype(np.float16)
            .reshape(128, -1)
        )
        xib_h = (
            xib_full[i * BC : (i + 1) * BC]
            .reshape(BC, NCH, 128, HWm)
            .transpose(2, 1, 0, 3)
            .astype(np.float16)
            .reshape(128, -1)
        )
        m = {
            "cst": np.ascontiguousarray(
                np.concatenate([aT_h, rb_h, id_h, xiq_h], axis=1)
            ),
            "cst2": np.ascontiguousarray(
                np.concatenate([xib_h, uT_h], axis=1)
            ),
            "x": np.ascontiguousarray(
                xc.reshape(BC, NCH, 128, L).transpose(0, 2, 1, 3).astype(np.float16)
            ),
            "xt": np.ascontiguousarray(
                xc.reshape(BC, C, NLV, LV).transpose(0, 3, 2, 1).astype(np.float16)
            ),
        }
        in_maps.append(m)
    return in_maps


def run(inputs, trace=False, tmpdir=None):
    from concourse.bass_utils import run_bass_kernel_spmd

    nc = _get_program()
    in_maps = make_in_maps(inputs)
    res = run_bass_kernel_spmd(
        nc, in_maps, core_ids=list(range(NCORES)), trace=trace, tmpdir=tmpdir
    )
    outs = np.stack([res.results[i]["out"] for i in range(NCORES)])  # (8,4,C,HW)
    full = (
        outs.reshape(B, C, HWm).reshape(B, C, 1, H, W).astype(np.float32)
    )
    return full, res


def kernel(**inputs):
    full, _ = run(inputs)
    return full


# revision 22
# speedup vs baseline: 1.1872x; 1.1872x over previous
"""Bass/Tile TRN2 kernel for nn_LocalNodeAttentionHead.

Reference computation (per sample b):
    xi = x[:, :, t0]  (center frame)          (C, HW)
    xw = x reshaped                           (C, L)    L = T*H*W
    q  = Wq @ xi + bq                         (CI, HW)
    k  = Wk @ xw + bk                         (CI, L)
    v  = Wv @ xw + bv                         (L, CI)
    S  = q^T k  -> softmax over L             (HW, L)
    y  = softmax(S) @ v                       (CI, HW)
    out = Wo @ y + bo + xi                    (C, HW)

Algebraic restructure (host-folded weights; softmax is the only
nonlinearity, everything around it is linear):
    A  = Wq^T Wk   (C x C)     S = (A^T xi + r)^T xw,  r = Wk^T bq
    U  = Wo Wv     (C x C)     out = U (xw P^T) + (bo + Wo bv) + xi
This removes the k- and v-projections entirely (462M MACs each per
sample); bk drops by softmax shift invariance, bv/bo fold into the
residual carrier.

All device matmuls run in float16 (full PE rate, no free-dim
constraint); accumulation is fp32 in PSUM. The softmax 1/rowsum is
folded into the P^T transpose by multiplying against diag(rinv)
instead of the identity.

Distribution: pure data-parallel, 4 samples per core on 8 cores.
Scores run one sample ahead of transpose/z so the PE never waits on
the softmax chain. DMA triggers cost ~0.7us each on the queue engine
and per-core HBM bandwidth is ~300GB/s, so transfers are merged into
one trigger per tensor per sample, all on the otherwise-idle sync
queue, ordered so each sample's data lands just in time. Elementwise
work is balanced across scalar (exp, PSUM drains) and vector
(reduce/diag/residual/drains).
"""

import sys

sys.path.insert(0, "/opt/trn_rl_repo")

import numpy as np

import concourse.bass as bass
import concourse.tile as tile
from concourse import bacc, mybir

F32 = mybir.dt.float32
F16 = mybir.dt.float16
AF = mybir.ActivationFunctionType

B, C, T, H, W = 32, 512, 9, 14, 14
HWm = H * W  # 196
L = T * HWm  # 1764
CENT = (T // 2) * HWm  # 784, center-frame offset in L
NCORES = 8
BC = B // NCORES  # 4 samples per core

NCH = C // 128  # 4 chunks of the channel dim
LK = 441  # l-chunk for scores (4 chunks, <=512 moving free dim)
NLK = L // LK
LV = 126  # l-chunk for P^T / attention sum (14 chunks, <=128 partitions)
NLV = L // LV
MC = 98  # query-row chunk (2 chunks of HW=196)
NMC = HWm // MC
X0A = 980  # first x0 piece [0:980) covers the center frame and lc 0,1


def build_program():
    nc = bacc.Bacc("TRN2", target_bir_lowering=False, debug=False)

    # host-pre-tiled inputs; each load is a single fully-contiguous DMA.
    # Small constants are concatenated per-partition into single tensors so
    # one trigger covers each group:
    #   cst  = aT (2048) | rb-as-f16 (8) | ident (128) | xiq (3136)
    #   cst2 = xib (3136) | uT (2048)
    CST = 2048 + 8 + 128 + 3136
    CST2 = 3136 + 2048
    x = nc.dram_tensor("x", [BC, 128, NCH, L], F16, kind="ExternalInput").ap()
    xt = nc.dram_tensor("xt", [BC, LV, NLV, C], F16, kind="ExternalInput").ap()
    cst = nc.dram_tensor("cst", [128, CST], F16, kind="ExternalInput").ap()
    cst2 = nc.dram_tensor("cst2", [128, CST2], F16, kind="ExternalInput").ap()
    out = nc.dram_tensor("out", [BC, C, HWm], F16, kind="ExternalOutput").ap()

    with tile.TileContext(nc) as tc:
        with (
            tc.tile_pool(name="const", bufs=1) as const,
            tc.tile_pool(name="sb", bufs=1) as sb,
            tc.tile_pool(name="ps", bufs=8, space="PSUM") as ps,
        ):
            # ---- constants (q'-path first so the PE starts immediately) ----
            cst_sb = const.tile([128, CST], F16)
            nc.sync.dma_start(cst_sb[:], cst[:])
            aT_sb = cst_sb[:, 0:2048].rearrange("p (j c) -> p j c", j=NCH)
            rb_sb = cst_sb[:, 2048:2056].bitcast(F32)  # [128, NCH] f32
            id_sb = cst_sb[:, 2056:2184]
            xi_sb = cst_sb[:, 2184:CST].rearrange(
                "p (j s m) -> p j s m", j=NCH, s=BC
            )
            cst2_sb = const.tile([128, CST2], F16)
            xib_sb = cst2_sb[:, 0:3136].rearrange(
                "p (j s m) -> p j s m", j=NCH, s=BC
            )
            uT_sb = cst2_sb[:, 3136:CST2].rearrange("p (j c) -> p j c", j=NCH)
            q_sb = const.tile([128, NCH, BC * HWm], F16)

            def emit_qproj(pair):
                # q' for a sample pair, batched to N=392
                for ci in range(NCH):
                    qp = ps.tile([128, 2 * HWm], F32, tag="ps", name="qp")
                    for j in range(NCH):
                        nc.tensor.matmul(
                            qp[:],
                            aT_sb[:, j, ci * 128 : (ci + 1) * 128],
                            xi_sb[:, j, 2 * pair : 2 * pair + 2, :],
                            start=(j == 0),
                            stop=(j == NCH - 1),
                        )
                    nc.scalar.activation(
                        q_sb[
                            :, ci, 2 * pair * HWm : 2 * (pair + 1) * HWm
                        ],
                        qp[:],
                        AF.Identity,
                        bias=rb_sb[:, ci : ci + 1],
                    )

            # ---- per-sample pieces -----------------------------------------
            xw_t, xt_t, st_t, pt_t, rs_t, z_t = {}, {}, {}, {}, {}, {}

            def emit_load_x(s):
                xw = sb.tile([128, NCH, L], F16, tag="xw", bufs=2, name="xw")
                xw_t[s] = xw
                if s == 0:
                    # first piece covers the center frame (q') and lc 0,1
                    nc.sync.dma_start(xw[:, :, 0:X0A], x[s][:, :, 0:X0A])
                    nc.sync.dma_start(xw[:, :, X0A:L], x[s][:, :, X0A:L])
                else:
                    nc.sync.dma_start(xw[:], x[s])

            def emit_load_xt(s):
                xts = sb.tile([128, NLV, C], F16, tag="xt", bufs=2, name="xt")
                xt_t[s] = xts
                nc.sync.dma_start(xts[0:LV, :, :], xt[s])

            cmax_t = {}

            def emit_scores(s, midA=None, midB=None):
                # chunked=True (last sample): per-chunk PSUM reduces so the
                # row max is ready right after the last score chunk, keeping
                # the tail's softmax latency off the critical path
                chunked = s == BC - 1
                xw = xw_t[s]
                st = [
                    sb.tile([MC, L], F32, tag=f"s{mc}", bufs=2, name=f"s{mc}")
                    for mc in range(NMC)
                ]
                st_t[s] = st
                if chunked:
                    cmax_t[s] = [
                        sb.tile([MC, NLK], F32, tag=f"cm{mc}", bufs=1, name=f"cm{mc}")
                        for mc in range(NMC)
                    ]
                for lc in range(NLK):
                    if lc == 0 and midA is not None:
                        midA()
                    if lc == 1 and midB is not None:
                        midB()
                    for mc in range(NMC):
                        sp = ps.tile([MC, LK], F32, tag="ps", name="sp")
                        for cj in range(NCH):
                            nc.tensor.matmul(
                                sp[:],
                                q_sb[
                                    :,
                                    cj,
                                    s * HWm + mc * MC : s * HWm + (mc + 1) * MC,
                                ],
                                xw[:, cj, lc * LK : (lc + 1) * LK],
                                start=(cj == 0),
                                stop=(cj == NCH - 1),
                            )
                        if chunked:
                            nc.scalar.copy(
                                st[mc][:, lc * LK : (lc + 1) * LK], sp[:]
                            )
                            nc.vector.reduce_max(
                                cmax_t[s][mc][:, lc : lc + 1],
                                sp[:],
                                axis=mybir.AxisListType.X,
                            )
                        elif lc < 2:
                            # early chunks drain on vector so the scalar
                            # queue is free for the previous sample's exp
                            nc.vector.tensor_copy(
                                st[mc][:, lc * LK : (lc + 1) * LK], sp[:]
                            )
                        else:
                            nc.scalar.copy(
                                st[mc][:, lc * LK : (lc + 1) * LK], sp[:]
                            )

            nm_t = {}

            def emit_negmax(s):
                # row max on vector (chunked variant reduces the tiny cmax)
                chunked = s == BC - 1
                nm = []
                for mc in range(NMC):
                    negmax = sb.tile([MC, 1], F32, tag="negmax", bufs=2, name="negmax")
                    nc.vector.reduce_max(
                        negmax[:],
                        cmax_t[s][mc][:] if chunked else st_t[s][mc][:],
                        axis=mybir.AxisListType.X,
                        negate=True,
                    )
                    nm.append(negmax)
                nm_t[s] = nm

            def emit_exp(s):
                # exp (unnormalized, max-shifted) -> f16 P with the row sum
                # accumulated
                pt, rs = [], []
                for mc in range(NMC):
                    p = sb.tile([MC, L], F16, tag=f"p{mc}", bufs=2, name=f"p{mc}")
                    rsum = sb.tile([MC, 1], F32, tag="rsum", bufs=2, name="rsum")
                    nc.scalar.activation(
                        p[:],
                        st_t[s][mc][:],
                        AF.Exp,
                        bias=nm_t[s][mc][:],
                        accum_out=rsum[:],
                    )
                    pt.append(p)
                    rs.append(rsum)
                pt_t[s] = pt
                rs_t[s] = rs

            def emit_tz(s):
                # P^T (normalized via diag(rinv)) and z = xw P^T, streamed
                # over l-chunk pairs; z accumulates C-major so no output
                # transpose is needed
                pt, xts = pt_t[s], xt_t[s]
                dg = []
                for mc in range(NMC):
                    rinv = sb.tile([MC, 1], F32, tag="rinv", bufs=2, name="rinv")
                    nc.vector.reciprocal(rinv[:], rs_t[s][mc][:])
                    d = sb.tile([MC, MC], F16, tag=f"dg{mc}", bufs=2, name=f"dg{mc}")
                    nc.vector.tensor_scalar_mul(d[:], id_sb[0:MC, 0:MC], rinv[:])
                    dg.append(d)
                z_ps = [
                    ps.tile([128, HWm], F32, tag="ps", name=f"z{cc}")
                    for cc in range(NCH)
                ]
                for lp in range(NLV // 2):
                    ptp = ps.tile([LV, 2 * HWm], F32, tag="ps", name="ptp")
                    for half in range(2):
                        lc = 2 * lp + half
                        for mc in range(NMC):
                            nc.tensor.matmul(
                                ptp[
                                    :,
                                    half * HWm + mc * MC : half * HWm + (mc + 1) * MC,
                                ],
                                pt[mc][:, lc * LV : (lc + 1) * LV],
                                dg[mc][:],
                                start=True,
                                stop=True,
                            )
                    ptsb = sb.tile(
                        [128, 2 * HWm], F16, tag="ptsb", bufs=3, name="ptsb"
                    )
                    if lp % 2 == 0:
                        nc.vector.tensor_copy(ptsb[0:LV, :], ptp[:])
                    else:
                        nc.scalar.copy(ptsb[0:LV, :], ptp[:])
                    for half in range(2):
                        lc = 2 * lp + half
                        for cc in range(NCH):
                            nc.tensor.matmul(
                                z_ps[cc][:],
                                xts[0:LV, lc, cc * 128 : (cc + 1) * 128],
                                ptsb[0:LV, half * HWm : (half + 1) * HWm],
                                start=(lc == 0),
                                stop=(lc == NLV - 1),
                            )
                # z -> sbuf f16 for the output projection (pair-batched)
                if s % 2 == 0:
                    z_t[s // 2] = sb.tile(
                        [128, NCH, 2 * HWm], F16, tag="z2", bufs=2, name="z2"
                    )
                z2 = z_t[s // 2]
                for cc in range(NCH):
                    nc.vector.tensor_copy(
                        z2[:, cc, (s % 2) * HWm : (s % 2 + 1) * HWm], z_ps[cc][:]
                    )

            def emit_out(pair):
                # output projection + residual (center frame + bo2, f16 const)
                z2 = z_t[pair]
                osb = sb.tile([128, NCH, 2 * HWm], F16, tag="osb", bufs=2, name="osb")
                for co in range(NCH):
                    op = ps.tile([128, 2 * HWm], F32, tag="ps", name="op")
                    for cj in range(NCH):
                        nc.tensor.matmul(
                            op[:],
                            uT_sb[:, cj, co * 128 : (co + 1) * 128],
                            z2[:, cj, :],
                            start=(cj == 0),
                            stop=(cj == NCH - 1),
                        )
                    nc.vector.tensor_add(
                        osb[:, co, :],
                        op[:],
                        xib_sb[:, co, 2 * pair : 2 * pair + 2, :],
                    )
                for ds in range(2):
                    nc.sync.dma_start(
                        out[2 * pair + ds].rearrange("(j p) m -> p j m", p=128),
                        osb[:, :, ds * HWm : (ds + 1) * HWm],
                    )

            # ---- schedule: scores one sample ahead of transpose/z; DMA
            # triggers ordered so each sample's data lands just in time -----
            emit_load_x(0)
            emit_load_x(1)
            emit_qproj(0)
            emit_qproj(1)
            emit_scores(0)
            emit_negmax(0)
            emit_exp(0)
            emit_load_xt(0)

            emit_scores(1)
            emit_load_x(2)
            emit_load_xt(1)
            emit_tz(0)

            nc.sync.dma_start(cst2_sb[:], cst2[:])
            emit_scores(
                2,
                midA=lambda: emit_negmax(1),
                midB=lambda: emit_exp(1),
            )
            emit_load_x(3)
            emit_load_xt(2)
            emit_tz(1)
            emit_out(0)

            emit_scores(
                3,
                midA=lambda: emit_negmax(2),
                midB=lambda: emit_exp(2),
            )
            emit_load_xt(3)
            emit_negmax(3)
            emit_exp(3)
            emit_tz(2)

            emit_tz(3)
            emit_out(1)

    nc.compile()
    return nc


_NC = None


def _get_program():
    global _NC
    if _NC is None:
        _NC = build_program()
    return _NC


def make_in_maps(inputs):
    x_window = np.ascontiguousarray(np.asarray(inputs["x_window"], dtype=np.float32))
    Wq = np.asarray(inputs["Wq"], dtype=np.float32)
    bq_ = np.asarray(inputs["bq"], dtype=np.float32)
    Wk = np.asarray(inputs["Wk"], dtype=np.float32)
    Wv = np.asarray(inputs["Wv"], dtype=np.float32)
    bv_ = np.asarray(inputs["bv"], dtype=np.float32)
    Wo = np.asarray(inputs["Wo"], dtype=np.float32)
    bo_ = np.asarray(inputs["bo"], dtype=np.float32)

    # host-folded weights (fp32, exact)
    A = Wq.T @ Wk  # (C, C): S = (A^T xi + r)^T xw
    r = Wk.T @ bq_
    U = Wo @ Wv  # (C, C): out = U (xw P^T) + bo2 + xi
    bo2 = bo_ + Wo @ bv_

    xw = x_window.reshape(B, C, L)
    # residual carrier: center frame + folded output bias
    xib_full = xw[:, :, CENT : CENT + HWm] + bo2[None, :, None]

    def tile_w(wt):  # (in, out) -> [128, NCH, out] partition-major
        return np.ascontiguousarray(
            wt.reshape(NCH, 128, -1).transpose(1, 0, 2).astype(np.float16)
        )

    aT_h = tile_w(A).reshape(128, -1)
    uT_h = tile_w(U.T).reshape(128, -1)
    rb_h = np.ascontiguousarray(
        r.reshape(NCH, 128).T.astype(np.float32)
    ).view(np.float16)  # [128, 8]
    id_h = np.eye(128, dtype=np.float16)
    in_maps = []
    for i in range(NCORES):
        xc = xw[i * BC : (i + 1) * BC]  # (BC, C, L)
        xiq_h = (
            xc[:, :, CENT : CENT + HWm]
            .reshape(BC, NCH, 128, HWm)
            .transpose(2, 1, 0, 3)
            .astype(np.float16)
            .reshape(128, -1)
        )
        xib_h = (
            xib_full[i * BC : (i + 1) * BC]
            .reshape(BC, NCH, 128, HWm)
            .transpose(2, 1, 0, 3)
            .astype(np.float16)
            .reshape(128, -1)
        )
        m = {
            "cst": np.ascontiguousarray(
                np.concatenate([aT_h, rb_h, id_h, xiq_h], axis=1)
            ),
            "cst2": np.ascontiguousarray(
                np.concatenate([xib_h, uT_h], axis=1)
            ),
            "x": np.ascontiguousarray(
                xc.reshape(BC, NCH, 128, L).transpose(0, 2, 1, 3).astype(np.float16)
            ),
            "xt": np.ascontiguousarray(
                xc.reshape(BC, C, NLV, LV).transpose(0, 3, 2, 1).astype(np.float16)
            ),
        }
        in_maps.append(m)
    return in_maps


def run(inputs, trace=False, tmpdir=None):
    from concourse.bass_utils import run_bass_kernel_spmd

    nc = _get_program()
    in_maps = make_in_maps(inputs)
    res = run_bass_kernel_spmd(
        nc, in_maps, core_ids=list(range(NCORES)), trace=trace, tmpdir=tmpdir
    )
    outs = np.stack([res.results[i]["out"] for i in range(NCORES)])  # (8,4,C,HW)
    full = (
        outs.reshape(B, C, HWm).reshape(B, C, 1, H, W).astype(np.float32)
    )
    return full, res


def kernel(**inputs):
    full, _ = run(inputs)
    return full
